# revision 7
# baseline (speedup 1.0000x reference)
"""Trainium2 Bass kernel for BigramKLLoss.

topk_sum[k] = sum_{b,t} probs[b,t,a_k] * probs[b,t+1,b_k] * pair_mask[b,t]
then a tiny KL finalize.

Strategy (8 NeuronCores): shard batch b 4-ways x pair-list K 2-ways.
Host pre-transposes probs[b] to a (V, T) bf16 row-major buffer, so each
pair needs two contiguous 2KB rows. On device, gpsimd dma_gather fetches
512 rows (1MB) per instruction into SBUF (pair index -> partition), and
one DVE affine_mul_reduce per 128 pairs computes
dots[p] = sum_t A[p,t]*B[p,t+1] in a single fused pass (f32 accumulate).
Partial dot products are summed across the 4 batch shards and finalized
on the host (O(K) scalar math).
"""

import math
from contextlib import ExitStack

import numpy as np
import ml_dtypes

import concourse.bacc as bacc
import concourse.bass as bass
import concourse.mybir as mybir
from concourse.bass_utils import run_bass_kernel_spmd
from concourse.library_config import mlp

# problem constants (hardcoded per harness contract)
B, T, V, K = 4, 1024, 32000, 50000
EPS_T, EPS_M = 1e-8, 1e-12

N_CORES = 8
BG = 4                    # batch shards
KG = N_CORES // BG        # pair-list shards
KPC = K // KG             # pairs per core (25000)
CHUNK = 512               # indices per dma_gather (1MB per gather)
SUB = CHUNK // 128        # affine_mul_reduce calls per chunk
NCHUNK = math.ceil(KPC / CHUNK)
KPAD = NCHUNK * CHUNK
NBUF = 4                  # gather buffering depth
IDXW = CHUNK // 16        # idx columns per chunk in the packed idx tensor

_nc_cache = {}


def _build_nc(masked: bool, repeat: int = 1):
    """Build the per-core Bass module (identical on all cores; SPMD)."""
    nc = bacc.Bacc("TRN2")
    dt = mybir.dt

    pt_a = nc.dram_tensor("pt_a", [V, T], dt.bfloat16, kind="ExternalInput")
    if masked:
        pt_b = nc.dram_tensor("pt_b", [V, T], dt.bfloat16, kind="ExternalInput")
    else:
        pt_b = pt_a
    ia = nc.dram_tensor("ia", [128, NCHUNK * IDXW], dt.int16, kind="ExternalInput")
    ib = nc.dram_tensor("ib", [128, NCHUNK * IDXW], dt.int16, kind="ExternalInput")
    dots = nc.dram_tensor(
        "dots", [128, NCHUNK * SUB], dt.float32, kind="ExternalOutput"
    )

    NG = repeat * NCHUNK  # total gather rounds

    with (
        ExitStack() as stack,
        nc.Block() as block,
        nc.sbuf_tensor("ia_s", [128, NCHUNK * IDXW], dt.int16) as ia_s,
        nc.sbuf_tensor("ib_s", [128, NCHUNK * IDXW], dt.int16) as ib_s,
        nc.sbuf_tensor("atile", [128, NBUF * SUB, T], dt.bfloat16) as atile,
        nc.sbuf_tensor("btile", [128, NBUF * SUB, T], dt.bfloat16) as btile,
        nc.sbuf_tensor("prod", [128, NBUF * SUB, T - 1], dt.bfloat16) as prod,
        nc.sbuf_tensor("dots_s", [128, NCHUNK * SUB], dt.float32) as dots_s,
        nc.semaphore("idx_sem") as idx_sem,
        nc.semaphore("out_sem") as out_sem,
    ):
        gsemA = [stack.enter_context(nc.semaphore(f"gA{s}")) for s in range(NBUF)]
        gsemB = [stack.enter_context(nc.semaphore(f"gB{s}")) for s in range(NBUF)]
        vsem = [stack.enter_context(nc.semaphore(f"v{s}")) for s in range(NBUF)]

        rounds_per_slot = [len(range(s, NG, NBUF)) for s in range(NBUF)]

        @block.sync
        def _(sync):
            sync.dma_start(ia_s[:], ia[:]).then_inc(idx_sem, 16)
            sync.dma_start(ib_s[:], ib[:]).then_inc(idx_sem, 16)
            for s in range(NBUF):
                sync.wait_ge(vsem[s], SUB * rounds_per_slot[s])
            sync.dma_start(dots[:], dots_s[:]).then_inc(out_sem, 16)
            sync.wait_ge(out_sem, 16)

        @block.gpsimd
        def _(g):
            g.load_library(mlp)
            g.wait_ge(idx_sem, 32)
            for glob in range(NG):
                ci = glob % NCHUNK
                s = glob % NBUF
                r = glob // NBUF
                if r >= 1:
                    g.wait_ge(vsem[s], SUB * r)
                g.dma_gather(
                    atile[:, s * SUB : (s + 1) * SUB, :],
                    pt_a[:],
                    ia_s[:, ci * IDXW : (ci + 1) * IDXW],
                    CHUNK,
                    CHUNK,
                    T,
                ).then_inc(gsemA[s], 16)
                g.dma_gather(
                    btile[:, s * SUB : (s + 1) * SUB, :],
                    pt_b[:],
                    ib_s[:, ci * IDXW : (ci + 1) * IDXW],
                    CHUNK,
                    CHUNK,
                    T,
                ).then_inc(gsemB[s], 16)

        @block.vector
        def _(v):
            for glob in range(NG):
                ci = glob % NCHUNK
                s = glob % NBUF
                r = glob // NBUF
                v.wait_ge(gsemA[s], 16 * (r + 1))
                v.wait_ge(gsemB[s], 16 * (r + 1))
                for j in range(SUB):
                    col = ci * SUB + j
                    v.affine_mul_reduce(
                        out=prod[:, s * SUB + j, :],
                        accum_out=dots_s[:, col : col + 1],
                        in0=atile[:, s * SUB + j, 0 : T - 1],
                        in1=btile[:, s * SUB + j, 1:T],
                        scale=1.0,
                        bias=0.0,
                    ).then_inc(vsem[s], 1)

    nc.compile()
    return nc


def _get_nc(masked: bool, repeat: int = 1):
    key = (masked, repeat)
    if key not in _nc_cache:
        _nc_cache[key] = _build_nc(masked, repeat)
    return _nc_cache[key]


def _bf16_transpose(u16_b, out=None):
    """(T, V) uint16 -> (V, T) contiguous uint16, blocked for cache."""
    if out is None:
        out = np.empty((u16_b.shape[1], u16_b.shape[0]), dtype=np.uint16)
    BS = 2048
    for v0 in range(0, u16_b.shape[1], BS):
        v1 = min(v0 + BS, u16_b.shape[1])
        out[v0:v1, :] = u16_b[:, v0:v1].T
    return out


def _pack_idxs(idx):
    """(KPAD,) int16 -> (128, NCHUNK*IDXW) packed+replicated for dma_gather."""
    arr = idx.reshape(NCHUNK, IDXW, 16)           # [chunk, col, p]
    slab = arr.transpose(2, 0, 1).reshape(16, NCHUNK * IDXW)
    return np.ascontiguousarray(np.tile(slab, (8, 1)))


def _prep_in_maps(probs, mask, pairs):
    """Host prep: per-core input maps. Returns (in_maps, masked, n_pairs)."""
    probs = np.ascontiguousarray(probs, dtype=np.float32)
    mask = np.asarray(mask)
    pairs = np.asarray(pairs)

    pair_mask = (mask[:, :-1] & mask[:, 1:])
    n_pairs = float(pair_mask.sum())
    masked = not bool(mask.all())

    u16 = probs.view(np.uint16)[..., 1::2]        # (B, T, V) truncated bf16
    pt_list = [_bf16_transpose(u16[b]) for b in range(B)]

    if masked:
        pmask = np.zeros((B, T), dtype=np.float32)
        pmask[:, : T - 1] = pair_mask.astype(np.float32)
        pa_list = []
        for b in range(B):
            masked_probs = np.ascontiguousarray(probs[b] * pmask[b][:, None])
            mu16 = masked_probs.view(np.uint16)[..., 1::2]
            pa_list.append(_bf16_transpose(mu16))
    else:
        pa_list = pt_list

    a_all = pairs[:, 0].astype(np.int16)
    b_all = pairs[:, 1].astype(np.int16)
    in_maps = []
    for c in range(N_CORES):
        bg = c % BG
        kg = c // BG
        a = np.zeros(KPAD, dtype=np.int16)
        b = np.zeros(KPAD, dtype=np.int16)
        a[:KPC] = a_all[kg * KPC : (kg + 1) * KPC]
        b[:KPC] = b_all[kg * KPC : (kg + 1) * KPC]
        m = {
            "pt_a": pa_list[bg].view(ml_dtypes.bfloat16),
            "ia": _pack_idxs(a),
            "ib": _pack_idxs(b),
        }
        if masked:
            m["pt_b"] = pt_list[bg].view(ml_dtypes.bfloat16)
        in_maps.append(m)
    return in_maps, masked, n_pairs


def _reduce_results(results):
    """Per-core dots -> topk_sum (K,) float64."""
    topk = np.zeros(K, dtype=np.float64)
    for c in range(N_CORES):
        kg = c // BG
        dots = np.asarray(results[c]["dots"])     # (128, NCHUNK*SUB) f32
        vals = dots.T.reshape(-1)[:KPC]           # pair i = col*128 + p
        topk[kg * KPC : (kg + 1) * KPC] += vals.astype(np.float64)
    return topk


def _finalize(topk, n_pairs, target_probs, target_oov):
    n = max(n_pairs, 1.0)
    model_top = np.maximum(topk / n, EPS_M)
    model_oov = float(np.clip(1.0 - model_top.sum(), EPS_M, 1.0 - EPS_T))
    tgt = np.maximum(np.asarray(target_probs, dtype=np.float64), EPS_T)
    t_oov = max(float(np.asarray(target_oov)[0]), EPS_T)
    kl_top = (model_top * (np.log(model_top) - np.log(tgt))).sum()
    kl_oov = model_oov * (np.log(model_oov) - math.log(t_oov))
    return np.float32(kl_top + kl_oov)


def kernel(probs, target_probs, target_oov, mask, pairs):
    in_maps, masked, n_pairs = _prep_in_maps(probs, mask, pairs)
    nc = _get_nc(masked)
    res = run_bass_kernel_spmd(nc, in_maps, core_ids=list(range(N_CORES)))
    topk = _reduce_results(res.results)
    return _finalize(topk, n_pairs, target_probs, target_oov)


# revision 10
# speedup vs baseline: 3.6646x; 3.6646x over previous
"""Trainium2 Bass kernel for BigramKLLoss.

topk_sum[k] = sum_{b,t} probs[b,t,a_k] * probs[b,t+1,b_k] * pair_mask[b,t]
then a tiny KL finalize.

Strategy (8 NeuronCores): shard batch b 4-ways x pair-list K 2-ways.
Host pre-transposes probs[b] to a (V, T) bf16 row-major buffer, so each
pair needs two contiguous 2KB rows. On device, gpsimd dma_gather fetches
512 rows (1MB) per instruction into SBUF (pair index -> partition), and
one DVE affine_mul_reduce per 128 pairs computes
dots[p] = sum_t A[p,t]*B[p,t+1] in a single fused pass (f32 accumulate).
Partial dot products are summed across the 4 batch shards and finalized
on the host (O(K) scalar math).
"""

import math
from contextlib import ExitStack

import numpy as np
import ml_dtypes

import concourse.bacc as bacc
import concourse.bass as bass
import concourse.mybir as mybir
from concourse.bass_utils import run_bass_kernel_spmd
from concourse.library_config import mlp

# problem constants (hardcoded per harness contract)
B, T, V, K = 4, 1024, 32000, 50000
EPS_T, EPS_M = 1e-8, 1e-12

N_CORES = 8
BG = 4                    # batch shards
KG = N_CORES // BG        # pair-list shards
KPC = K // KG             # pairs per core (25000)
CHUNK = 512               # indices per dma_gather (1MB per gather)
SUB = CHUNK // 128        # affine_mul_reduce calls per chunk
NCHUNK = math.ceil(KPC / CHUNK)
KPAD = NCHUNK * CHUNK
NBUF = 4                  # gather buffering depth
IDXW = CHUNK // 16        # idx columns per chunk in the packed idx tensor

_nc_cache = {}


def _build_nc(masked: bool, repeat: int = 1, variant: str = "full"):
    """Build the per-core Bass module (identical on all cores; SPMD).

    variant: "full" | "gather" (DMA only) | "compute" (DVE only)
    """
    do_gather = variant in ("full", "gather")
    do_compute = variant in ("full", "compute")
    nc = bacc.Bacc("TRN2")
    dt = mybir.dt

    pt_a = nc.dram_tensor("pt_a", [V, T], dt.bfloat16, kind="ExternalInput")
    if masked:
        pt_b = nc.dram_tensor("pt_b", [V, T], dt.bfloat16, kind="ExternalInput")
    else:
        pt_b = pt_a
    ia = nc.dram_tensor("ia", [128, NCHUNK * IDXW], dt.int16, kind="ExternalInput")
    ib = nc.dram_tensor("ib", [128, NCHUNK * IDXW], dt.int16, kind="ExternalInput")
    dots = nc.dram_tensor(
        "dots", [128, NCHUNK * SUB], dt.float32, kind="ExternalOutput"
    )

    NG = repeat * NCHUNK  # total gather rounds

    with (
        ExitStack() as stack,
        nc.Block() as block,
        nc.sbuf_tensor("ia_s", [128, NCHUNK * IDXW], dt.int16) as ia_s,
        nc.sbuf_tensor("ib_s", [128, NCHUNK * IDXW], dt.int16) as ib_s,
        nc.sbuf_tensor("atile", [128, NBUF * SUB, T], dt.bfloat16) as atile,
        nc.sbuf_tensor("btile", [128, NBUF * SUB, T], dt.bfloat16) as btile,
        nc.sbuf_tensor("prod", [128, NBUF * SUB, T - 1], dt.bfloat16) as prod,
        nc.sbuf_tensor("dots_s", [128, NCHUNK * SUB], dt.float32) as dots_s,
        nc.semaphore("idx_sem") as idx_sem,
        nc.semaphore("out_sem") as out_sem,
    ):
        gsemA = [stack.enter_context(nc.semaphore(f"gA{s}")) for s in range(NBUF)]
        gsemB = [stack.enter_context(nc.semaphore(f"gB{s}")) for s in range(NBUF)]
        vsem = [stack.enter_context(nc.semaphore(f"v{s}")) for s in range(NBUF)]

        rounds_per_slot = [len(range(s, NG, NBUF)) for s in range(NBUF)]

        @block.sync
        def _(sync):
            sync.dma_start(ia_s[:], ia[:]).then_inc(idx_sem, 16)
            sync.dma_start(ib_s[:], ib[:]).then_inc(idx_sem, 16)
            if do_compute:
                for s in range(NBUF):
                    sync.wait_ge(vsem[s], SUB * rounds_per_slot[s])
            else:
                for s in range(NBUF):
                    sync.wait_ge(gsemA[s], 16 * rounds_per_slot[s])
                    sync.wait_ge(gsemB[s], 16 * rounds_per_slot[s])
            sync.dma_start(dots[:], dots_s[:]).then_inc(out_sem, 16)
            sync.wait_ge(out_sem, 16)

        if do_gather:
            @block.gpsimd
            def _(g):
                g.load_library(mlp)
                g.wait_ge(idx_sem, 32)
                for glob in range(NG):
                    ci = glob % NCHUNK
                    s = glob % NBUF
                    r = glob // NBUF
                    if do_compute and r >= 1:
                        g.wait_ge(vsem[s], SUB * r)
                    g.dma_gather(
                        atile[:, s * SUB : (s + 1) * SUB, :],
                        pt_a[:],
                        ia_s[:, ci * IDXW : (ci + 1) * IDXW],
                        CHUNK,
                        CHUNK,
                        T,
                    ).then_inc(gsemA[s], 16)
                    g.dma_gather(
                        btile[:, s * SUB : (s + 1) * SUB, :],
                        pt_b[:],
                        ib_s[:, ci * IDXW : (ci + 1) * IDXW],
                        CHUNK,
                        CHUNK,
                        T,
                    ).then_inc(gsemB[s], 16)

        if do_compute:
            @block.vector
            def _(v):
                for glob in range(NG):
                    ci = glob % NCHUNK
                    s = glob % NBUF
                    r = glob // NBUF
                    if do_gather:
                        v.wait_ge(gsemA[s], 16 * (r + 1))
                        v.wait_ge(gsemB[s], 16 * (r + 1))
                    for j in range(SUB):
                        col = ci * SUB + j
                        v.affine_mul_reduce(
                            out=prod[:, s * SUB + j, :],
                            accum_out=dots_s[:, col : col + 1],
                            in0=atile[:, s * SUB + j, 0 : T - 1],
                            in1=btile[:, s * SUB + j, 1:T],
                            scale=1.0,
                            bias=0.0,
                        ).then_inc(vsem[s], 1)

    nc.compile()
    return nc


def _get_nc(masked: bool, repeat: int = 1, variant: str = "full"):
    key = (masked, repeat, variant, CHUNK, NBUF)
    if key not in _nc_cache:
        _nc_cache[key] = _build_nc(masked, repeat, variant)
    return _nc_cache[key]


def _bf16_transpose(u16_b, out=None):
    """(T, V) uint16 -> (V, T) contiguous uint16, blocked for cache."""
    if out is None:
        out = np.empty((u16_b.shape[1], u16_b.shape[0]), dtype=np.uint16)
    BS = 2048
    for v0 in range(0, u16_b.shape[1], BS):
        v1 = min(v0 + BS, u16_b.shape[1])
        out[v0:v1, :] = u16_b[:, v0:v1].T
    return out


def _pack_idxs(idx):
    """(KPAD,) int16 -> (128, NCHUNK*IDXW) packed+replicated for dma_gather."""
    arr = idx.reshape(NCHUNK, IDXW, 16)           # [chunk, col, p]
    slab = arr.transpose(2, 0, 1).reshape(16, NCHUNK * IDXW)
    return np.ascontiguousarray(np.tile(slab, (8, 1)))


def _prep_in_maps(probs, mask, pairs):
    """Host prep: per-core input maps. Returns (in_maps, masked, n_pairs)."""
    probs = np.ascontiguousarray(probs, dtype=np.float32)
    mask = np.asarray(mask)
    pairs = np.asarray(pairs)

    pair_mask = (mask[:, :-1] & mask[:, 1:])
    n_pairs = float(pair_mask.sum())
    masked = not bool(mask.all())

    u16 = probs.view(np.uint16)[..., 1::2]        # (B, T, V) truncated bf16
    pt_list = [_bf16_transpose(u16[b]) for b in range(B)]

    if masked:
        pmask = np.zeros((B, T), dtype=np.float32)
        pmask[:, : T - 1] = pair_mask.astype(np.float32)
        pa_list = []
        for b in range(B):
            masked_probs = np.ascontiguousarray(probs[b] * pmask[b][:, None])
            mu16 = masked_probs.view(np.uint16)[..., 1::2]
            pa_list.append(_bf16_transpose(mu16))
    else:
        pa_list = pt_list

    a_all = pairs[:, 0].astype(np.int16)
    b_all = pairs[:, 1].astype(np.int16)
    in_maps = []
    for c in range(N_CORES):
        bg = c % BG
        kg = c // BG
        a = np.zeros(KPAD, dtype=np.int16)
        b = np.zeros(KPAD, dtype=np.int16)
        a[:KPC] = a_all[kg * KPC : (kg + 1) * KPC]
        b[:KPC] = b_all[kg * KPC : (kg + 1) * KPC]
        m = {
            "pt_a": pa_list[bg].view(ml_dtypes.bfloat16),
            "ia": _pack_idxs(a),
            "ib": _pack_idxs(b),
        }
        if masked:
            m["pt_b"] = pt_list[bg].view(ml_dtypes.bfloat16)
        in_maps.append(m)
    return in_maps, masked, n_pairs


def _reduce_results(results):
    """Per-core dots -> topk_sum (K,) float64."""
    topk = np.zeros(K, dtype=np.float64)
    for c in range(N_CORES):
        kg = c // BG
        dots = np.asarray(results[c]["dots"])     # (128, NCHUNK*SUB) f32
        vals = dots.T.reshape(-1)[:KPC]           # pair i = col*128 + p
        topk[kg * KPC : (kg + 1) * KPC] += vals.astype(np.float64)
    return topk


def _finalize(topk, n_pairs, target_probs, target_oov):
    n = max(n_pairs, 1.0)
    model_top = np.maximum(topk / n, EPS_M)
    model_oov = float(np.clip(1.0 - model_top.sum(), EPS_M, 1.0 - EPS_T))
    tgt = np.maximum(np.asarray(target_probs, dtype=np.float64), EPS_T)
    t_oov = max(float(np.asarray(target_oov)[0]), EPS_T)
    kl_top = (model_top * (np.log(model_top) - np.log(tgt))).sum()
    kl_oov = model_oov * (np.log(model_oov) - math.log(t_oov))
    return np.float32(kl_top + kl_oov)


def kernel(probs, target_probs, target_oov, mask, pairs):
    in_maps, masked, n_pairs = _prep_in_maps(probs, mask, pairs)
    nc = _get_nc(masked)
    res = run_bass_kernel_spmd(nc, in_maps, core_ids=list(range(N_CORES)))
    topk = _reduce_results(res.results)
    return _finalize(topk, n_pairs, target_probs, target_oov)
